# revision 19
# baseline (speedup 1.0000x reference)
"""Adaptive 7x7 Gaussian filter (softmax tap weights) on 8 TRN2 NeuronCores.

Math: per pixel, tap weight at offset (dr,dc) is softmax over 49 taps of
-((dr^2+dc^2)/2)*sigma^2.  With t = exp(-sigma^2/2) the unnormalized
weight is t^(dr^2+dc^2); the normalizer factorizes: Z = g^2 with
g = 1 + 2t + 2t^4 + 2t^9.  Grouping taps by squared distance
d in {0,1,2,4,5,8,9,10,13,18} with binary-stencil sums S_d and
u=t, v=t^4, w=t^9 (the ten weights are {1,u,u^2,v,uv,v^2,w,uw,vw,w^2}):

  out = (S0 + u*(S1 + u*S2)
             + v*(S4 + u*S5 + v*S8)
             + w*(S9 + u*S10 + v*S13 + w*S18)) * exp(-2*ln(g))

S_d decompose into vertical pair sums V_a (a=1..3) and horizontal pair
sums C_{a,b} of those (plus of x itself), all free-dim-shifted adds.

Layout per core: 2 images x 3 channels = 6 planes of 256x256; each
plane split into 16 bands of 16 rows -> 96 partitions.  A partition
holds its band padded to [22 rows x 262 cols] (3 halo rows, reflect
cols), so BOTH stencil directions are free-dim AP offsets.  Compute in
fp16 (DVE 2x dual-pump, ~1.4e-3 end-to-end error), exp/ln on ScalarE.
"""

import sys

sys.path.insert(0, "/opt/trn_rl_repo")

import numpy as np

import concourse.bacc as bacc
import concourse.bass as bass
import concourse.mybir as mybir
import concourse.tile as tile
from concourse.bass_utils import run_bass_kernel_spmd

B, CH, H, W = 16, 3, 256, 256
N_CORES = 8
B_PER_CORE = B // N_CORES          # 2 images per core
PLANES = B_PER_CORE * CH           # 6 planes per core
BANDS = 16                         # bands per plane
BR = H // BANDS                    # 16 rows per band
P = PLANES * BANDS                 # 96 partitions
PADR = BR + 6                      # 22 padded rows
PADC = W + 6                       # 262 padded cols
FMAP = BR * W                      # 4096 elems per partition per map

F16 = mybir.dt.float16
F32 = mybir.dt.float32
AF = mybir.ActivationFunctionType


def build_nc():
    nc = bacc.Bacc(None, target_bir_lowering=False)
    x_d = nc.declare_dram_parameter("x", [B_PER_CORE, CH, H, W], F32, isOutput=False)
    s_d = nc.declare_dram_parameter("sigma", [B_PER_CORE, CH, H, W], F32, isOutput=False)
    o_d = nc.declare_dram_parameter("out", [B_PER_CORE, CH, H, W], F32, isOutput=True)
    # halo matrices, stacked [96, 4*96] = [top-shift, top-self, bot-shift,
    # bot-self].  out[m] = sum_k lhsT[k, m] rhs[k]:
    #   top-shift[k, m] = 1 iff m%16 != 0 and k == m-1
    #   top-self [k, m] = 1 iff m%16 == 0 and k == m
    #   bot-shift[k, m] = 1 iff m%16 != 15 and k == m+1
    #   bot-self [k, m] = 1 iff m%16 == 15 and k == m
    sh_d = nc.declare_dram_parameter("hshift", [P, 4 * P], F16, isOutput=False)

    # DRAM views: [96 partitions, 16 rows, 256 cols]
    xv = x_d[:].rearrange("b ch (bd r) c -> (b ch bd) r c", r=BR)
    sv = s_d[:].rearrange("b ch (bd r) c -> (b ch bd) r c", r=BR)
    ov = o_d[:].rearrange("b ch (bd r) c -> (b ch bd) r c", r=BR)

    with tile.TileContext(nc) as tc:
        with (
            tc.tile_pool(name="io", bufs=2) as io,
            tc.tile_pool(name="xp", bufs=1) as xpp,
            tc.tile_pool(name="maps", bufs=7) as maps,
            tc.tile_pool(name="vp", bufs=3) as vp,
            tc.tile_pool(name="work", bufs=6) as work,
        ):
            sf32 = io.tile([P, FMAP], F32, tag="iof32", name="sf32")

            xpad = xpp.tile([P, PADR * PADC], F16)
            xpv = xpad[:].rearrange("p (r c) -> p r c", r=PADR)

            def nt(name):
                return maps.tile([P, FMAP], F16, tag="mp", name=name)

            s2, u, v, w = nt("s2"), nt("u"), nt("v"), nt("w")
            gs, rz, acc, t1 = nt("gs"), nt("rz"), nt("acc"), nt("t1")

            def r3(t):
                return t[:].rearrange("p (r c) -> p r c", r=BR)

            # ---- loads ----
            # All x-side DMAs go through the single SWDGE queue (gpsimd):
            # one completion semaphore and FIFO ordering among them, so no
            # instruction ever needs >1 sync wait.
            xf32 = io.tile([P, FMAP], F32, tag="iof32", name="xf32")
            nc.gpsimd.dma_start(out=r3(xf32), in_=xv)
            nc.sync.dma_start(out=r3(sf32), in_=sv)

            # ---- cast main rows into padded interior (ACT) ----
            nc.scalar.copy(out=xpv[:, 3 : 3 + BR, 3 : 3 + W], in_=r3(xf32))

            # ---- column reflect pads, main rows (ACT, neg-step) ----
            nc.scalar.copy(out=xpv[:, 3 : 3 + BR, 0:3], in_=xpv[:, 3 : 3 + BR, 6:3:-1])
            nc.scalar.copy(
                out=xpv[:, 3 : 3 + BR, 259:262], in_=xpv[:, 3 : 3 + BR, 257:254:-1]
            )

            # ---- self-reflect halo rows on ALL partitions (ACT, full width).
            # Correct for plane-edge bands; interior bands overwritten below.
            nc.scalar.copy(out=xpv[:, 0:3, :], in_=xpv[:, 6:3:-1, :])
            nc.scalar.copy(out=xpv[:, 19:22, :], in_=xpv[:, 17:14:-1, :])

            # ---- interior halo exchange on TensorE ----
            # Interior bands pull the neighbor band's edge rows; plane-edge
            # bands pass through their own (already correct) self-reflect
            # halo rows.  Keeps xpad written exclusively by the ACT stream.
            shmat = xpp.tile([P, 4 * P], F16, name="shmat")
            nc.gpsimd.dma_start(out=shmat[:], in_=sh_d[:])
            with tc.tile_pool(name="psum", bufs=2, space="PSUM") as psp:
                for i, (dst, src_n) in enumerate(
                    (((0, 3), (16, 19)), ((19, 22), (3, 6)))
                ):
                    m_shift = shmat[:, (2 * i) * P : (2 * i + 1) * P]
                    m_self = shmat[:, (2 * i + 1) * P : (2 * i + 2) * P]
                    ps = psp.tile([P, 1024], F32, tag="ps", name=f"ps{i}")
                    rflat = xpv[:, src_n[0] : src_n[1], :].rearrange("p r c -> p (r c)")
                    sflat = xpv[:, dst[0] : dst[1], :].rearrange("p r c -> p (r c)")
                    for n0, n1 in ((0, 512), (512, 786)):
                        nc.tensor.matmul(
                            ps[:, n0:n1], m_shift, rflat[:, n0:n1], start=True, stop=False
                        )
                        nc.tensor.matmul(
                            ps[:, n0:n1], m_self, sflat[:, n0:n1], start=False, stop=True
                        )
                    nc.scalar.copy(
                        out=xpv[:, dst[0] : dst[1], :].rearrange("p r c -> p (r c)"),
                        in_=ps[:, 0:786],
                    )

            # ---- sigma -> s2; u,v,w = t^1, t^4, t^9 ----
            nc.scalar.activation(s2[:], sf32[:], AF.Square)
            nc.scalar.activation(u[:], s2[:], AF.Exp, scale=-0.5)
            nc.scalar.activation(v[:], s2[:], AF.Exp, scale=-2.0)
            nc.scalar.activation(w[:], s2[:], AF.Exp, scale=-4.5)

            # ---- normalizer: rZ = exp(-2 ln(1 + 2(u+v+w))) ----
            lng = io.tile([P, FMAP], F32, tag="iof32", name="lng")
            nc.vector.tensor_add(gs[:], u[:], v[:])
            nc.vector.tensor_add(gs[:], gs[:], w[:])
            nc.scalar.activation(lng[:], gs[:], AF.Ln, bias=1.0, scale=2.0)
            nc.scalar.activation(rz[:], lng[:], AF.Exp, scale=-2.0)

            # ---- stencils + combine, lifetime-interleaved ----
            def vmap(a):
                """V_a = vertical pair-sum over full padded width: [P,BR,PADC]."""
                t = vp.tile([P, BR * PADC], F16, tag="vt", name=f"v{a}")
                tv = t[:].rearrange("p (r c) -> p r c", r=BR)
                nc.vector.tensor_add(
                    tv, xpv[:, 3 - a : 3 - a + BR, :], xpv[:, 3 + a : 3 + a + BR, :]
                )
                return tv

            _wn = [0]

            def cmap(src3, b):
                """C = horizontal pair-sum of a [P,BR,PADC] view -> [P,FMAP] tile."""
                _wn[0] += 1
                out = work.tile([P, FMAP], F16, tag="wk", name=f"w{_wn[0]}")
                nc.vector.tensor_add(
                    r3(out), src3[:, :, 3 - b : 3 - b + W], src3[:, :, 3 + b : 3 + b + W]
                )
                return out

            def swk():
                _wn[0] += 1
                return work.tile([P, FMAP], F16, tag="wk", name=f"w{_wn[0]}")

            X3 = xpv[:, 3 : 3 + BR, :]  # x as a [P,BR,PADC] view

            V3 = vmap(3)
            V2 = vmap(2)
            V1 = vmap(1)

            # --- block3 = S9 + u*S10 + v*S13 + w*S18; acc = w*block3 ---
            C33 = cmap(V3, 3)                                  # S18
            nc.vector.tensor_mul(acc[:], w[:], C33[:])
            C32 = cmap(V3, 2)
            C23 = cmap(V2, 3)
            S13 = swk()
            nc.vector.tensor_add(S13[:], C23[:], C32[:])
            nc.vector.tensor_mul(t1[:], v[:], S13[:])
            nc.vector.tensor_add(acc[:], acc[:], t1[:])
            C31 = cmap(V3, 1)
            C13 = cmap(V1, 3)
            S10 = swk()
            nc.vector.tensor_add(S10[:], C13[:], C31[:])
            nc.vector.tensor_mul(t1[:], u[:], S10[:])
            nc.vector.tensor_add(acc[:], acc[:], t1[:])
            C03 = cmap(X3, 3)
            S9 = swk()
            nc.vector.tensor_add(r3(S9), r3(C03), V3[:, :, 3 : 3 + W])
            nc.vector.tensor_add(acc[:], acc[:], S9[:])
            nc.vector.tensor_mul(acc[:], acc[:], w[:])

            # --- block2 = S4 + u*S5 + v*S8; acc += v*block2 ---
            C22 = cmap(V2, 2)                                  # S8
            t2 = swk()
            nc.vector.tensor_mul(t2[:], v[:], C22[:])
            C12 = cmap(V1, 2)
            C21 = cmap(V2, 1)
            S5 = swk()
            nc.vector.tensor_add(S5[:], C12[:], C21[:])
            nc.vector.tensor_mul(t1[:], u[:], S5[:])
            nc.vector.tensor_add(t2[:], t2[:], t1[:])
            C02 = cmap(X3, 2)
            S4 = swk()
            nc.vector.tensor_add(r3(S4), r3(C02), V2[:, :, 3 : 3 + W])
            nc.vector.tensor_add(t2[:], t2[:], S4[:])
            nc.vector.tensor_mul(t2[:], t2[:], v[:])
            nc.vector.tensor_add(acc[:], acc[:], t2[:])

            # --- block1 = u*(S1 + u*S2); acc += block1 + S0 ---
            C11 = cmap(V1, 1)                                  # S2
            nc.vector.tensor_mul(t1[:], u[:], C11[:])
            C01 = cmap(X3, 1)
            S1 = swk()
            nc.vector.tensor_add(r3(S1), r3(C01), V1[:, :, 3 : 3 + W])
            nc.vector.tensor_add(t1[:], t1[:], S1[:])
            nc.vector.tensor_mul(t1[:], t1[:], u[:])
            nc.vector.tensor_add(acc[:], acc[:], t1[:])
            nc.vector.tensor_add(r3(acc), r3(acc), xpv[:, 3 : 3 + BR, 3 : 3 + W])

            # ---- final: out = acc * rZ; cast to f32; store ----
            obf = nt("obf")
            nc.vector.tensor_mul(obf[:], acc[:], rz[:])
            outf = io.tile([P, FMAP], F32, tag="iof32", name="outf")
            nc.scalar.copy(out=outf[:], in_=obf[:])
            nc.gpsimd.dma_start(out=ov, in_=r3(outf))

    nc.compile()
    return nc


def make_in_maps(x, sigma):
    sh = np.zeros((P, 4 * P), np.float16)
    for m in range(P):
        if m % BANDS != 0:
            sh[m - 1, m] = 1.0          # top-shift
        else:
            sh[m, P + m] = 1.0          # top-self
        if m % BANDS != BANDS - 1:
            sh[m + 1, 2 * P + m] = 1.0  # bot-shift
        else:
            sh[m, 3 * P + m] = 1.0      # bot-self
    return [
        {
            "x": x[i * B_PER_CORE : (i + 1) * B_PER_CORE],
            "sigma": sigma[i * B_PER_CORE : (i + 1) * B_PER_CORE],
            "hshift": sh,
        }
        for i in range(N_CORES)
    ]


_NC_CACHE = None


def kernel(x: np.ndarray, sigma: np.ndarray) -> np.ndarray:
    global _NC_CACHE
    x = np.ascontiguousarray(np.asarray(x, dtype=np.float32))
    sigma = np.ascontiguousarray(np.asarray(sigma, dtype=np.float32))
    if _NC_CACHE is None:
        _NC_CACHE = build_nc()
    nc = _NC_CACHE
    in_maps = make_in_maps(x, sigma)
    res = run_bass_kernel_spmd(nc, in_maps, core_ids=list(range(N_CORES)))
    outs = [res.results[i]["out"] for i in range(N_CORES)]
    return np.concatenate(outs, axis=0).astype(np.float32)


# revision 20
# speedup vs baseline: 1.2431x; 1.2431x over previous
"""Adaptive 7x7 Gaussian filter (softmax tap weights) on 8 TRN2 NeuronCores.

Math: per pixel, tap weight at offset (dr,dc) is softmax over 49 taps of
-((dr^2+dc^2)/2)*sigma^2.  With t = exp(-sigma^2/2) the unnormalized
weight is t^(dr^2+dc^2); the normalizer factorizes: Z = g^2 with
g = 1 + 2t + 2t^4 + 2t^9.  Grouping taps by squared distance
d in {0,1,2,4,5,8,9,10,13,18} with binary-stencil sums S_d and
u=t, v=t^4, w=t^9 (the ten weights are {1,u,u^2,v,uv,v^2,w,uw,vw,w^2}):

  out = (S0 + u*(S1 + u*S2)
             + v*(S4 + u*S5 + v*S8)
             + w*(S9 + u*S10 + v*S13 + w*S18)) * exp(-2*ln(g))

S_d decompose into vertical pair sums V_a (a=1..3) and horizontal pair
sums C_{a,b} of those (plus of x itself), all free-dim-shifted adds.

Layout per core: 2 images x 3 channels = 6 planes of 256x256; each
plane split into 16 bands of 16 rows -> 96 partitions.  A partition
holds its band padded to [22 rows x 262 cols] (3 halo rows, reflect
cols), so BOTH stencil directions are free-dim AP offsets.  Compute in
fp16 (DVE 2x dual-pump, ~1.4e-3 end-to-end error), exp/ln on ScalarE.
"""

import sys

sys.path.insert(0, "/opt/trn_rl_repo")

import numpy as np

import concourse.bacc as bacc
import concourse.bass as bass
import concourse.mybir as mybir
import concourse.tile as tile
from concourse.bass_utils import run_bass_kernel_spmd

B, CH, H, W = 16, 3, 256, 256
N_CORES = 8
B_PER_CORE = B // N_CORES          # 2 images per core
PLANES = B_PER_CORE * CH           # 6 planes per core
BANDS = 16                         # bands per plane
BR = H // BANDS                    # 16 rows per band
P = PLANES * BANDS                 # 96 partitions
PADR = BR + 6                      # 22 padded rows
PADC = W + 6                       # 262 padded cols
FMAP = BR * W                      # 4096 elems per partition per map

F16 = mybir.dt.float16
F32 = mybir.dt.float32
AF = mybir.ActivationFunctionType


def build_nc():
    nc = bacc.Bacc(None, target_bir_lowering=False)
    x_d = nc.declare_dram_parameter("x", [B_PER_CORE, CH, H, W], F32, isOutput=False)
    s_d = nc.declare_dram_parameter("sigma", [B_PER_CORE, CH, H, W], F32, isOutput=False)
    o_d = nc.declare_dram_parameter("out", [B_PER_CORE, CH, H, W], F32, isOutput=True)
    # halo matrices, stacked [96, 4*96] = [top-shift, top-self, bot-shift,
    # bot-self].  out[m] = sum_k lhsT[k, m] rhs[k]:
    #   top-shift[k, m] = 1 iff m%16 != 0 and k == m-1
    #   top-self [k, m] = 1 iff m%16 == 0 and k == m
    #   bot-shift[k, m] = 1 iff m%16 != 15 and k == m+1
    #   bot-self [k, m] = 1 iff m%16 == 15 and k == m
    sh_d = nc.declare_dram_parameter("hshift", [P, 4 * P], F16, isOutput=False)

    # DRAM views: [96 partitions, 16 rows, 256 cols]
    xv = x_d[:].rearrange("b ch (bd r) c -> (b ch bd) r c", r=BR)
    sv = s_d[:].rearrange("b ch (bd r) c -> (b ch bd) r c", r=BR)
    ov = o_d[:].rearrange("b ch (bd r) c -> (b ch bd) r c", r=BR)

    with tile.TileContext(nc) as tc:
        with (
            tc.tile_pool(name="io", bufs=2) as io,
            tc.tile_pool(name="xp", bufs=1) as xpp,
            tc.tile_pool(name="maps", bufs=7) as maps,
            tc.tile_pool(name="vp", bufs=3) as vp,
            tc.tile_pool(name="work", bufs=6) as work,
        ):
            sf32 = io.tile([P, FMAP], F32, tag="iof32", name="sf32")

            xpad = xpp.tile([P, PADR * PADC], F16)
            xpv = xpad[:].rearrange("p (r c) -> p r c", r=PADR)

            def nt(name):
                return maps.tile([P, FMAP], F16, tag="mp", name=name)

            s2, u, v, w = nt("s2"), nt("u"), nt("v"), nt("w")
            gs, rz, acc, t1 = nt("gs"), nt("rz"), nt("acc"), nt("t1")

            def r3(t):
                return t[:].rearrange("p (r c) -> p r c", r=BR)

            # ---- loads ----
            # All x-side DMAs go through the single SWDGE queue (gpsimd):
            # one completion semaphore and FIFO ordering among them, so no
            # instruction ever needs >1 sync wait.
            xf32 = io.tile([P, FMAP], F32, tag="iof32", name="xf32")
            nc.sync.dma_start(out=r3(xf32), in_=xv)
            nc.sync.dma_start(out=r3(sf32), in_=sv)

            # ---- cast main rows into padded interior (ACT) ----
            nc.scalar.copy(out=xpv[:, 3 : 3 + BR, 3 : 3 + W], in_=r3(xf32))

            # ---- column reflect pads, main rows (ACT, neg-step) ----
            nc.scalar.copy(out=xpv[:, 3 : 3 + BR, 0:3], in_=xpv[:, 3 : 3 + BR, 6:3:-1])
            nc.scalar.copy(
                out=xpv[:, 3 : 3 + BR, 259:262], in_=xpv[:, 3 : 3 + BR, 257:254:-1]
            )

            # ---- self-reflect halo rows on ALL partitions (ACT, full width).
            # Correct for plane-edge bands; interior bands overwritten below.
            nc.scalar.copy(out=xpv[:, 0:3, :], in_=xpv[:, 6:3:-1, :])
            nc.scalar.copy(out=xpv[:, 19:22, :], in_=xpv[:, 17:14:-1, :])

            # ---- interior halo exchange on TensorE ----
            # Interior bands pull the neighbor band's edge rows; plane-edge
            # bands pass through their own (already correct) self-reflect
            # halo rows.  Keeps xpad written exclusively by the ACT stream.
            shmat = xpp.tile([P, 4 * P], F16, name="shmat")
            nc.sync.dma_start(out=shmat[:], in_=sh_d[:])
            with tc.tile_pool(name="psum", bufs=2, space="PSUM") as psp:
                for i, (dst, src_n) in enumerate(
                    (((0, 3), (16, 19)), ((19, 22), (3, 6)))
                ):
                    m_shift = shmat[:, (2 * i) * P : (2 * i + 1) * P]
                    m_self = shmat[:, (2 * i + 1) * P : (2 * i + 2) * P]
                    ps = psp.tile([P, 1024], F32, tag="ps", name=f"ps{i}")
                    rflat = xpv[:, src_n[0] : src_n[1], :].rearrange("p r c -> p (r c)")
                    sflat = xpv[:, dst[0] : dst[1], :].rearrange("p r c -> p (r c)")
                    for n0, n1 in ((0, 512), (512, 786)):
                        nc.tensor.matmul(
                            ps[:, n0:n1], m_shift, rflat[:, n0:n1], start=True, stop=False
                        )
                        nc.tensor.matmul(
                            ps[:, n0:n1], m_self, sflat[:, n0:n1], start=False, stop=True
                        )
                    nc.scalar.copy(
                        out=xpv[:, dst[0] : dst[1], :].rearrange("p r c -> p (r c)"),
                        in_=ps[:, 0:786],
                    )

            # ---- sigma -> s2; u,v,w = t^1, t^4, t^9 ----
            nc.scalar.activation(s2[:], sf32[:], AF.Square)
            nc.scalar.activation(u[:], s2[:], AF.Exp, scale=-0.5)
            nc.scalar.activation(v[:], s2[:], AF.Exp, scale=-2.0)
            nc.scalar.activation(w[:], s2[:], AF.Exp, scale=-4.5)

            # ---- normalizer: rZ = exp(-2 ln(1 + 2(u+v+w))) ----
            lng = io.tile([P, FMAP], F32, tag="iof32", name="lng")
            nc.vector.tensor_add(gs[:], u[:], v[:])
            nc.vector.tensor_add(gs[:], gs[:], w[:])
            nc.scalar.activation(lng[:], gs[:], AF.Ln, bias=1.0, scale=2.0)
            nc.scalar.activation(rz[:], lng[:], AF.Exp, scale=-2.0)

            # ---- stencils + combine, lifetime-interleaved ----
            def vmap(a):
                """V_a = vertical pair-sum over full padded width: [P,BR,PADC]."""
                t = vp.tile([P, BR * PADC], F16, tag="vt", name=f"v{a}")
                tv = t[:].rearrange("p (r c) -> p r c", r=BR)
                nc.vector.tensor_add(
                    tv, xpv[:, 3 - a : 3 - a + BR, :], xpv[:, 3 + a : 3 + a + BR, :]
                )
                return tv

            _wn = [0]

            def cmap(src3, b):
                """C = horizontal pair-sum of a [P,BR,PADC] view -> [P,FMAP] tile."""
                _wn[0] += 1
                out = work.tile([P, FMAP], F16, tag="wk", name=f"w{_wn[0]}")
                nc.vector.tensor_add(
                    r3(out), src3[:, :, 3 - b : 3 - b + W], src3[:, :, 3 + b : 3 + b + W]
                )
                return out

            def swk():
                _wn[0] += 1
                return work.tile([P, FMAP], F16, tag="wk", name=f"w{_wn[0]}")

            X3 = xpv[:, 3 : 3 + BR, :]  # x as a [P,BR,PADC] view

            V3 = vmap(3)
            V2 = vmap(2)
            V1 = vmap(1)

            # --- block3 = S9 + u*S10 + v*S13 + w*S18; acc = w*block3 ---
            C33 = cmap(V3, 3)                                  # S18
            nc.vector.tensor_mul(acc[:], w[:], C33[:])
            C32 = cmap(V3, 2)
            C23 = cmap(V2, 3)
            S13 = swk()
            nc.vector.tensor_add(S13[:], C23[:], C32[:])
            nc.vector.tensor_mul(t1[:], v[:], S13[:])
            nc.vector.tensor_add(acc[:], acc[:], t1[:])
            C31 = cmap(V3, 1)
            C13 = cmap(V1, 3)
            S10 = swk()
            nc.vector.tensor_add(S10[:], C13[:], C31[:])
            nc.vector.tensor_mul(t1[:], u[:], S10[:])
            nc.vector.tensor_add(acc[:], acc[:], t1[:])
            C03 = cmap(X3, 3)
            S9 = swk()
            nc.vector.tensor_add(r3(S9), r3(C03), V3[:, :, 3 : 3 + W])
            nc.vector.tensor_add(acc[:], acc[:], S9[:])
            nc.vector.tensor_mul(acc[:], acc[:], w[:])

            # --- block2 = S4 + u*S5 + v*S8; acc += v*block2 ---
            C22 = cmap(V2, 2)                                  # S8
            t2 = swk()
            nc.vector.tensor_mul(t2[:], v[:], C22[:])
            C12 = cmap(V1, 2)
            C21 = cmap(V2, 1)
            S5 = swk()
            nc.vector.tensor_add(S5[:], C12[:], C21[:])
            nc.vector.tensor_mul(t1[:], u[:], S5[:])
            nc.vector.tensor_add(t2[:], t2[:], t1[:])
            C02 = cmap(X3, 2)
            S4 = swk()
            nc.vector.tensor_add(r3(S4), r3(C02), V2[:, :, 3 : 3 + W])
            nc.vector.tensor_add(t2[:], t2[:], S4[:])
            nc.vector.tensor_mul(t2[:], t2[:], v[:])
            nc.vector.tensor_add(acc[:], acc[:], t2[:])

            # --- block1 = u*(S1 + u*S2); acc += block1 + S0 ---
            C11 = cmap(V1, 1)                                  # S2
            nc.vector.tensor_mul(t1[:], u[:], C11[:])
            C01 = cmap(X3, 1)
            S1 = swk()
            nc.vector.tensor_add(r3(S1), r3(C01), V1[:, :, 3 : 3 + W])
            nc.vector.tensor_add(t1[:], t1[:], S1[:])
            nc.vector.tensor_mul(t1[:], t1[:], u[:])
            nc.vector.tensor_add(acc[:], acc[:], t1[:])
            nc.vector.tensor_add(r3(acc), r3(acc), xpv[:, 3 : 3 + BR, 3 : 3 + W])

            # ---- final: out = acc * rZ; cast to f32; store ----
            obf = nt("obf")
            nc.vector.tensor_mul(obf[:], acc[:], rz[:])
            outf = io.tile([P, FMAP], F32, tag="iof32", name="outf")
            nc.scalar.copy(out=outf[:], in_=obf[:])
            nc.sync.dma_start(out=ov, in_=r3(outf))

    nc.compile()
    return nc


def make_in_maps(x, sigma):
    sh = np.zeros((P, 4 * P), np.float16)
    for m in range(P):
        if m % BANDS != 0:
            sh[m - 1, m] = 1.0          # top-shift
        else:
            sh[m, P + m] = 1.0          # top-self
        if m % BANDS != BANDS - 1:
            sh[m + 1, 2 * P + m] = 1.0  # bot-shift
        else:
            sh[m, 3 * P + m] = 1.0      # bot-self
    return [
        {
            "x": x[i * B_PER_CORE : (i + 1) * B_PER_CORE],
            "sigma": sigma[i * B_PER_CORE : (i + 1) * B_PER_CORE],
            "hshift": sh,
        }
        for i in range(N_CORES)
    ]


_NC_CACHE = None


def kernel(x: np.ndarray, sigma: np.ndarray) -> np.ndarray:
    global _NC_CACHE
    x = np.ascontiguousarray(np.asarray(x, dtype=np.float32))
    sigma = np.ascontiguousarray(np.asarray(sigma, dtype=np.float32))
    if _NC_CACHE is None:
        _NC_CACHE = build_nc()
    nc = _NC_CACHE
    in_maps = make_in_maps(x, sigma)
    res = run_bass_kernel_spmd(nc, in_maps, core_ids=list(range(N_CORES)))
    outs = [res.results[i]["out"] for i in range(N_CORES)]
    return np.concatenate(outs, axis=0).astype(np.float32)
